# revision 31
# baseline (speedup 1.0000x reference)
"""Binarized dense layer (tanh(sign(x) @ sign(w) + b)) on 8 Trainium2 cores.

Full input shapes (hardcoded): inputs [8192, 4096] f32, kernel [4096, 4096] f32,
bias [4096] f32 -> out [8192, 4096] f32.

Sharding: 4 batch shards x 2 output-column shards (core i -> r=i//2, c=i%2).
Per core: x [2048, 4096], w [4096, 2048], b [2048] -> y [2048, 2048].

Wire format: inputs are shipped as bf16 (sign-preserving for all normal
floats; the binarize consumes only the sign, computed on device), and the
X shard is laid out K-major (transposed) so the contraction dim lands on
SBUF partitions directly. All reference ops (binarize X, binarize W,
matmul, bias add, tanh) run on device.

Per-core kernel (Tile framework):
  - W: DMA bf16 row-chunks, binarize to +-1 fp8e4 on ACT (Sign), resident
    in SBUF as [128, 2, O] per 256-row K-pair for fp8 DoubleRow matmul.
  - X^T: DMA bf16 k-chunks [128, B], binarize to +-0.5 fp8e4 with
    tensor_scalar (is_ge 0.0, subtract 0.5) on DVE/GpSimd; the whole
    binarized X^T (8.4 MB fp8) stays resident in SBUF.
  - Matmul: fp8 DoubleRow, K=256 per step, N=512 (one PSUM bank), M=128.
    PSUM accumulates 0.5 * (+-1 dot) exactly in f32.
  - Streaming phase: the first m_split m-tiles run k < KP/2 while the
    second halves of W/X^T stream in; partials spill to SBUF as fp16
    (exact: values are n/2 with |n/2| <= 1024) and are added back later.
  - Output: tanh on ACT reading PSUM with scale=2.0 (exact: psum = S/2),
    f32 staged in SBUF, DMA out.
"""

import sys
import types

if "/opt/trn_rl_repo" not in sys.path:
    sys.path.insert(0, "/opt/trn_rl_repo")

from contextlib import ExitStack

import numpy as np
import ml_dtypes

import concourse.bass as bass
import concourse.tile as tile
from concourse import bacc, mybir


def _ensure_ntff_hook_module():
    """The RL image's antenv lacks axon_hooks, which bass_utils imports for
    trace=True under axon. Register a functional shim in sys.modules."""
    name = "antenv.axon_hooks"
    if name in sys.modules:
        return
    try:
        import antenv
        __import__(name)
        return  # real module exists
    except ImportError:
        pass
    mod = types.ModuleType(name)
    mod._hook = None

    def set_axon_ntff_profile_hook(hook):
        mod._hook = hook

    def get_axon_ntff_profile_hook():
        if mod._hook is None:
            try:
                from trn_agent_boot.trn_boot import _ntff_profile_via_ctypes
                mod._hook = _ntff_profile_via_ctypes("/opt/axon/libaxon_pjrt.so")
            except Exception:
                return None
        return mod._hook

    mod.set_axon_ntff_profile_hook = set_axon_ntff_profile_hook
    mod.get_axon_ntff_profile_hook = get_axon_ntff_profile_hook
    sys.modules[name] = mod
    try:
        import antenv
        antenv.axon_hooks = mod
    except ImportError:
        pass


_ensure_ntff_hook_module()

from concourse.bass_utils import run_bass_kernel_spmd  # noqa: E402

F32 = mybir.dt.float32
F16 = mybir.dt.float16
BF16 = mybir.dt.bfloat16
FP8 = mybir.dt.float8e4

N_CORES = 8
R_SHARDS = 4  # batch shards
C_SHARDS = 2  # output-column shards

B_FULL, D_FULL, O_FULL = 8192, 4096, 4096
B_LOC = B_FULL // R_SHARDS   # 2048
O_LOC = O_FULL // C_SHARDS   # 2048


def build_nc(b_loc=B_LOC, d=D_FULL, o_loc=O_LOC, bias_nonzero=False,
             m_split=10, warmers=True):
    """Build the per-core Bass program (identical across cores)."""
    assert b_loc % 128 == 0 and d % 256 == 0 and o_loc % 512 == 0
    M = b_loc // 128    # m-tiles
    KP = d // 256       # DoubleRow K-pairs
    KC = d // 128       # 128-row chunks of the contraction dim
    N = o_loc // 512    # n-tiles (one PSUM bank each)
    m_split = min(m_split, M)
    if KP < 2:
        m_split = 0
    KH = KP // 2
    KCH = KC // 2

    nc = bacc.Bacc("TRN2", target_bir_lowering=False, debug=False,
                   num_devices=N_CORES)
    # x is the TRANSPOSED shard: [d, b_loc], K-major
    x = nc.dram_tensor("x", [d, b_loc], BF16, kind="ExternalInput")
    w = nc.dram_tensor("w", [d, o_loc], BF16, kind="ExternalInput")
    b = nc.dram_tensor("b", [o_loc], F32, kind="ExternalInput")
    y = nc.dram_tensor("y", [b_loc, o_loc], F32, kind="ExternalOutput")

    with tile.TileContext(nc) as tc, ExitStack() as ctx:
        singles = ctx.enter_context(tc.tile_pool(name="singles", bufs=1))
        wstage = ctx.enter_context(tc.tile_pool(name="wstage", bufs=3))
        wbp = ctx.enter_context(tc.tile_pool(name="wb", bufs=KP))
        xstage = ctx.enter_context(tc.tile_pool(name="xs", bufs=2))
        xbtp = ctx.enter_context(tc.tile_pool(name="xbt", bufs=1))
        ostage = ctx.enter_context(tc.tile_pool(name="ost", bufs=4))
        partp = ctx.enter_context(tc.tile_pool(name="part",
                                               bufs=max(m_split, 1)))
        pacc = ctx.enter_context(tc.tile_pool(name="pa", bufs=7,
                                              space="PSUM"))
        pscr = ctx.enter_context(tc.tile_pool(name="pscr", bufs=1,
                                              space="PSUM"))
        scratch = (pscr.tile([128, 64], F32, name="scratch")
                   if warmers else None)

        bias_bc = None
        if bias_nonzero:
            bias_bc = singles.tile([128, o_loc], F32)
            bias_ap = bass.AP(tensor=b.ap().tensor, offset=0,
                              ap=[[0, 128], [1, o_loc]])
            nc.gpsimd.dma_start(out=bias_bc[:], in_=bias_ap)
            # psum holds S/2 and tanh applies scale=2.0, so add bias/2
            nc.vector.tensor_scalar_mul(bias_bc[:], bias_bc[:], 0.5)

        # whole binarized X^T stays resident: [128, KC, b_loc] fp8
        xbt = xbtp.tile([128, KC, b_loc], FP8)

        def load_x_chunk(c, engine, dma=None):
            xs = xstage.tile([128, b_loc], BF16, tag="xs", name=f"xs{c}")
            (dma or nc.gpsimd).dma_start(out=xs[:],
                                         in_=x[c * 128:(c + 1) * 128, :])
            # binarize to +-0.5 fp8 in one pass
            engine.tensor_scalar(
                out=xbt[:, c, :], in0=xs[:], scalar1=0.0, scalar2=0.5,
                op0=mybir.AluOpType.is_ge, op1=mybir.AluOpType.subtract)


        def load_w_pair(k):
            t = wbp.tile([128, 2, o_loc], FP8, tag="wb", name=f"wb{k}")
            for j in (0, 1):
                s = wstage.tile([128, o_loc], BF16, tag="ws", name=f"ws{k}_{j}")
                nc.sync.dma_start(
                    out=s[:], in_=w[(2 * k + j) * 128:(2 * k + j + 1) * 128, :])
                nc.scalar.activation(out=t[:, j, :], in_=s[:],
                                     func=mybir.ActivationFunctionType.Sign)
            return t

        # ---- first halves of X^T and W (X on gpsimd queue, W on sync) ----
        for c in range(KCH):
            load_x_chunk(c, nc.vector)
        wb = []
        for k in range(KH):
            wb.append(load_w_pair(k))

        def k_group(pa, m, k0, k1, warm=False):
            for k in range(k0, k1):
                lhsT = xbt[:, 2 * k:2 * k + 2, m * 128:(m + 1) * 128]
                for n in range(N):
                    nc.tensor.matmul(
                        pa[n][:], lhsT, wb[k][:, :, n * 512:(n + 1) * 512],
                        start=(k == k0), stop=(k == k1 - 1),
                        perf_mode=mybir.MatmulPerfMode.DoubleRow)
                if warm and scratch is not None and k < k1 - 1:
                    # tiny matmul paced like the next X chunk: splits the
                    # W-arrival-paced idle gap below the ~3.4us HAM window
                    # so the PE keeps its 2.4 GHz clock. Same-arrival dep as
                    # the surrounding matmuls, so it adds no serialization.
                    c = min(2 * k + 1, KC - 1)
                    nc.tensor.matmul(scratch[:], xbt[:, c, 0:128],
                                     xbt[:, c, 0:64], start=True, stop=True)

        def finish(m, pa):
            o = ostage.tile([128, o_loc // 2], F32, tag="o", name=f"o{m}")
            for half in range(2):
                for nn in range(N // 2):
                    n = half * (N // 2) + nn
                    pn = pa[n][:]
                    if bias_bc is not None:
                        nc.vector.tensor_tensor(
                            out=pn, in0=pn,
                            in1=bias_bc[:, n * 512:(n + 1) * 512],
                            op=mybir.AluOpType.add)
                    nc.scalar.activation(
                        out=o[:, nn * 512:(nn + 1) * 512], in_=pn,
                        func=mybir.ActivationFunctionType.Tanh, scale=2.0)
                nc.sync.dma_start(
                    out=y[m * 128:(m + 1) * 128,
                          half * (o_loc // 2):(half + 1) * (o_loc // 2)],
                    in_=o[:])
                if half == 0:
                    o = ostage.tile([128, o_loc // 2], F32, tag="o",
                                    name=f"o{m}b")

        def alloc_pa(m):
            return [pacc.tile([128, 512], F32, tag="pa", name=f"pa_{m}_{n}")
                    for n in range(N)]

        # ---- phase 1: first m_split tiles accumulate k < KH while the
        # second halves stream in; partials spill to SBUF as fp16; the
        # second-half X binarizes are interleaved on the DVE queue so they
        # don't block behind (or get blocked by) the partial evictions ----
        parts = {}
        c2 = KCH  # next second-half X chunk to emit
        k2 = KH   # next second-half W pair to emit
        pair = m_split >= 2 and N >= 2
        if pair:
            # m0 (all N banks) and m1 (N-1 banks) run phase 1 interleaved:
            # together they use all 7 pacc banks, doubling how fast the
            # W-paced stream window consumes each arriving chunk. m1's last
            # n-strip is covered by a full k-loop in phase 2 instead.
            pa0 = alloc_pa(0)
            pa1 = [pacc.tile([128, 512], F32, tag="pa", name=f"pa_1_{n}")
                   for n in range(N - 1)]
            for k in range(KH):
                for n in range(N):
                    nc.tensor.matmul(
                        pa0[n][:], xbt[:, 2 * k:2 * k + 2, 0:128],
                        wb[k][:, :, n * 512:(n + 1) * 512],
                        start=(k == 0), stop=(k == KH - 1),
                        perf_mode=mybir.MatmulPerfMode.DoubleRow)
                if scratch is not None and k < KH - 1:
                    c = min(2 * k + 1, KC - 1)
                    nc.tensor.matmul(scratch[:], xbt[:, c, 0:128],
                                     xbt[:, c, 0:64], start=True, stop=True)
                for n in range(N - 1):
                    nc.tensor.matmul(
                        pa1[n][:], xbt[:, 2 * k:2 * k + 2, 128:256],
                        wb[k][:, :, n * 512:(n + 1) * 512],
                        start=(k == 0), stop=(k == KH - 1),
                        perf_mode=mybir.MatmulPerfMode.DoubleRow)
            for m, pa in ((0, pa0), (1, pa1)):
                part = partp.tile([128, N, 512], F16, tag="part",
                                  name=f"part{m}")
                for n in range(len(pa)):
                    nc.vector.tensor_copy(out=part[:, n, :], in_=pa[n][:])
                parts[m] = part
        for m in range(2 if pair else 0, m_split):
            pa = alloc_pa(m)
            k_group(pa, m, 0, KH, warm=(m == 0))
            part = partp.tile([128, N, 512], F16, tag="part", name=f"part{m}")
            for n in range(N):
                nc.vector.tensor_copy(out=part[:, n, :], in_=pa[n][:])
            parts[m] = part
            # interleave a slice of the second-half loads
            for _ in range((KH + m_split - 1) // m_split):
                if k2 < KP:
                    wb.append(load_w_pair(k2))
                    k2 += 1
            for _ in range((KCH + m_split - 1) // m_split):
                if c2 < KC:
                    load_x_chunk(c2, nc.vector, dma=nc.sync)
                    c2 += 1
        while k2 < KP:
            wb.append(load_w_pair(k2))
            k2 += 1
        while c2 < KC:
            load_x_chunk(c2, nc.vector, dma=nc.sync)
            c2 += 1

        # ---- phase 2: finish the split tiles (k >= KH, add partial) ----
        for m in range(m_split):
            pa = alloc_pa(m)
            if pair and m == 1:
                # n < N-1 resumes from the fp16 partial; the last n-strip
                # had no phase-1 coverage and runs the full k loop here
                for k in range(KP):
                    lhsT = xbt[:, 2 * k:2 * k + 2, 128:256]
                    if k >= KH:
                        for n in range(N - 1):
                            nc.tensor.matmul(
                                pa[n][:], lhsT,
                                wb[k][:, :, n * 512:(n + 1) * 512],
                                start=(k == KH), stop=(k == KP - 1),
                                perf_mode=mybir.MatmulPerfMode.DoubleRow)
                    nn = N - 1
                    nc.tensor.matmul(
                        pa[nn][:], lhsT, wb[k][:, :, nn * 512:(nn + 1) * 512],
                        start=(k == 0), stop=(k == KP - 1),
                        perf_mode=mybir.MatmulPerfMode.DoubleRow)
                for n in range(N - 1):
                    nc.vector.tensor_tensor(out=pa[n][:], in0=pa[n][:],
                                            in1=parts[m][:, n, :],
                                            op=mybir.AluOpType.add)
            else:
                k_group(pa, m, KH, KP)
                for n in range(N):
                    nc.vector.tensor_tensor(out=pa[n][:], in0=pa[n][:],
                                            in1=parts[m][:, n, :],
                                            op=mybir.AluOpType.add)
            finish(m, pa)

        # ---- remaining m-tiles: single-pass k loop ----
        for m in range(m_split, M):
            pa = alloc_pa(m)
            k_group(pa, m, 0, KP)
            finish(m, pa)

    nc.compile()
    return nc


_NC_CACHE = {}


def _get_nc(key, **kwargs):
    if key not in _NC_CACHE:
        _NC_CACHE[key] = build_nc(**kwargs)
    return _NC_CACHE[key]


def kernel(inputs: np.ndarray, kernel: np.ndarray, bias: np.ndarray,
           _trace: bool = False, _trace_cores=None) -> np.ndarray:
    x = np.asarray(inputs, dtype=np.float32).astype(ml_dtypes.bfloat16)
    w = np.asarray(kernel, dtype=np.float32).astype(ml_dtypes.bfloat16)
    b = np.ascontiguousarray(bias, dtype=np.float32)
    assert x.shape == (B_FULL, D_FULL) and w.shape == (D_FULL, O_FULL)

    bias_nonzero = bool(np.any(b != 0))
    nc = _get_nc(("full", bias_nonzero), bias_nonzero=bias_nonzero,
                 m_split=8 if bias_nonzero else 10)

    in_maps = []
    for i in range(N_CORES):
        r, c = i // C_SHARDS, i % C_SHARDS
        in_maps.append({
            "x": np.ascontiguousarray(x[r * B_LOC:(r + 1) * B_LOC, :].T),
            "w": np.ascontiguousarray(w[:, c * O_LOC:(c + 1) * O_LOC]),
            "b": np.ascontiguousarray(b[c * O_LOC:(c + 1) * O_LOC]),
        })

    res = run_bass_kernel_spmd(nc, in_maps, list(range(N_CORES)),
                               trace=_trace, trace_cores=_trace_cores)

    out = np.empty((B_FULL, O_FULL), dtype=np.float32)
    for i in range(N_CORES):
        r, c = i // C_SHARDS, i % C_SHARDS
        out[r * B_LOC:(r + 1) * B_LOC, c * O_LOC:(c + 1) * O_LOC] = \
            res.results[i]["y"]

    if _trace:
        return out, res
    return out
